# revision 1
# baseline (speedup 1.0000x reference)
"""Bloom transformer block on 8 Trainium2 NeuronCores.

Comm-free sharding: core c handles batch c//4 and 512 of its 2048 tokens
(two causally-balanced 256-token q-tiles {r, r+4}, r = c%4).  Every core
redundantly computes LN1 + K/V projections for its full batch; Q/attention/
o_proj/MLP run only on its own tokens.  All rank-dependence lives in
host-prepared per-core data (token permutation + alibi/causal-mask tiles),
so the device program is identical across cores (SPMD), with no collectives.
The host concatenates the per-core output slices.

Exact math shortcuts: the k-projection bias adds a per-query constant to
every score row, so it cancels in softmax and is dropped; the v-projection
bias adds exactly bv to each attention output (probs sum to 1), so bv @ wo
is folded into the host-prepared residual.  LN affine params are folded
into the following matmul weights; 1/sqrt(HD) is folded into Wq.
"""

import math
import os

import numpy as np
import ml_dtypes

import concourse.bass as bass
import concourse.tile as tile
from concourse import mybir
from concourse.bass_utils import run_bass_kernel_spmd

B, S, D, H = 2, 2048, 2048, 16
HD = D // H          # 128
FF = 4 * D           # 8192
EPS = 1e-5
NCORES = 8
GS = 4               # cores per batch (group size)
QT = S // GS         # own tokens per core = 512
NEG = -1.0e9
QW = 256             # q-tile width (tokens) in attention
NSLOT = QT // QW     # 2 slots per core
# padded k-extent (in 128-token k-tiles) per slot: slot j covers orig q-tile
# i = r + 4j; causal needs 2(i+1) k-tiles; padded to max over r.
KEXT = [8, 16]
NKT = S // 128       # 16 k-tiles
DT16 = D // 128      # 16 feature tiles of 128
FT64 = FF // 128     # 64 FF tiles
QSCALE = 1.0 / math.sqrt(HD)

f32 = mybir.dt.float32
bf16 = mybir.dt.bfloat16


def _alibi_slopes(num_heads):
    closest = 2 ** math.floor(math.log2(num_heads))
    base = 2.0 ** (-(2.0 ** (-(math.log2(closest) - 3))))
    powers = np.arange(1, 1 + closest, dtype=np.float64)
    slopes = base ** powers
    if closest != num_heads:
        extra_base = 2.0 ** (-(2.0 ** (-(math.log2(2 * closest) - 3))))
        num_rem = min(closest, num_heads - closest)
        extra_powers = np.arange(1, 1 + 2 * num_rem, 2, dtype=np.float64)
        slopes = np.concatenate([slopes, extra_base ** extra_powers])
    return slopes.astype(np.float32)


# ---------------------------------------------------------------------------
# wait-split post-pass: this walrus build supports a single sync-wait per
# instruction; excess waits move onto preceding NoOps on the same engine.
# ---------------------------------------------------------------------------
_ctr = [0]


def _split_waits(nc, maxw=1):
    for f in nc.m.functions:
        for bb in f.blocks:
            out = []
            changed = False
            for ins in bb.instructions:
                si = ins.sync_info
                waits = list(si.on_wait) if (si and si.on_wait) else []
                if len(waits) > maxw:
                    head, keep = waits[:-maxw], waits[-maxw:]
                    for w in head:
                        _ctr[0] += 1
                        nop = mybir.InstNoOp(name=f"I-waitsplit-{_ctr[0]}")
                        nop.engine = ins.engine
                        nop.sync_info = mybir.SyncInfo(on_wait=[w], on_update=[])
                        out.append(nop)
                    si.on_wait = keep
                    changed = True
                out.append(ins)
            if changed:
                bb.instructions = out
    return nc


# ---------------------------------------------------------------------------
# per-rank host-side structure
# ---------------------------------------------------------------------------
def _rank_structure(r):
    own256 = [r, r + 4]
    own128 = []
    for t in own256:
        own128 += [2 * t, 2 * t + 1]
    others128 = [t for t in range(NKT) if t not in own128]
    perm128 = own128 + others128
    klists = []
    for j in range(NSLOT):
        nown = 2 * j + 2
        kl = own128[:nown] + others128[: KEXT[j] - nown]
        klists.append(kl)
    return own256, perm128, klists


def _build_acol(r, slopes):
    """Per-core [128, H*24] f32: column h*24 + off_j + p holds
    slope_h * (k0_orig(p) + kk - i*256), or NEG for fully-masked pad tiles.
    s_final = scores + acol - arow  ==  scores + slope*(k - q) (+mask)."""
    _, _, klists = _rank_structure(r)
    out = np.empty((128, H * sum(KEXT)), dtype=np.float32)
    kk = np.arange(128, dtype=np.float64)
    for h in range(H):
        for j in range(NSLOT):
            i = r + 4 * j
            off = h * sum(KEXT) + sum(KEXT[:j])
            for p, kt in enumerate(klists[j]):
                if kt >= 2 * i + 2:          # fully beyond causal (pad)
                    out[:, off + p] = NEG
                else:
                    out[:, off + p] = (slopes[h]
                                       * (kt * 128 + kk - i * QW))
    return out


def _build_arow(slopes):
    """Shared [H, 3, 128, QW] f32 row tiles (subtracted from scores):
    var 0 = plain slope_h*qq; var 1/2 = plain + 1e9 on the causally-masked
    cells of the two own-diagonal k-tiles (kk > qq, 128+kk > qq)."""
    out = np.empty((H, 3, 128, QW), dtype=np.float32)
    kk = np.arange(128)
    qq = np.arange(QW)
    m0 = (kk[:, None] > qq[None, :]).astype(np.float32) * (-NEG)
    m1 = ((kk[:, None] + 128) > qq[None, :]).astype(np.float32) * (-NEG)
    for h in range(H):
        plain = np.broadcast_to(slopes[h] * qq[None, :].astype(np.float32),
                                (128, QW))
        out[h, 0] = plain
        out[h, 1] = plain + m0
        out[h, 2] = plain + m1
    return out


# ---------------------------------------------------------------------------
# device program (identical for all cores)
# ---------------------------------------------------------------------------
def build_nc(debug=False):
    nc = bass.Bass(target_bir_lowering=False)

    xp = nc.dram_tensor("xp", [S, D], f32, kind="ExternalInput")
    xres = nc.dram_tensor("xres", [QT, D], f32, kind="ExternalInput")
    wqkv = nc.dram_tensor("wqkv", [D, 3 * D], bf16, kind="ExternalInput")
    bq_pp = nc.dram_tensor("bq_pp", [128, DT16], f32, kind="ExternalInput")
    wo = nc.dram_tensor("wo", [D, D], bf16, kind="ExternalInput")
    w1 = nc.dram_tensor("w1", [D, FF], bf16, kind="ExternalInput")
    b1_pp = nc.dram_tensor("b1_pp", [128, FT64], f32, kind="ExternalInput")
    w2 = nc.dram_tensor("w2", [FF, D], bf16, kind="ExternalInput")
    b2_bc = nc.dram_tensor("b2_bc", [128, D], f32, kind="ExternalInput")
    acol = nc.dram_tensor("acol", [128, H * sum(KEXT)], f32,
                          kind="ExternalInput")
    arow = nc.dram_tensor("arow", [H, 3, 128, QW], f32, kind="ExternalInput")

    out = nc.dram_tensor("out", [QT, D], f32, kind="ExternalOutput")

    ikind = "ExternalOutput" if debug else "Internal"
    h_dram = nc.dram_tensor("h_dram", [S, D], bf16, kind=ikind)
    kT_dram = nc.dram_tensor("kT_dram", [D, S], bf16, kind=ikind)
    v_dram = nc.dram_tensor("v_dram", [S, D], bf16, kind=ikind)
    x2_dram = nc.dram_tensor("x2_dram", [QT, D], f32, kind=ikind)
    h2_dram = nc.dram_tensor("h2_dram", [QT, D], bf16, kind=ikind)
    rec_dram = nc.dram_tensor("rec_dram", [NSLOT * H, QW], f32, kind="Internal")

    with tile.TileContext(nc) as tc:
        with tc.tile_pool(name="persist", bufs=1) as pp:
            ones = pp.tile([128, 1], bf16, tag="ones")
            nc.vector.memset(ones, 1.0)
            eps_t = pp.tile([128, 1], f32, tag="eps")
            nc.vector.memset(eps_t, EPS)
            b1_sb = pp.tile([128, FT64], f32, tag="b1")
            nc.sync.dma_start(out=b1_sb, in_=b1_pp.ap())
            qT = [pp.tile([128, QT], bf16, tag=f"qT{m}", name=f"qT{m}")
                  for m in range(DT16)]

            _phase1(nc, tc, xp, h_dram, kT_dram, v_dram, wqkv, bq_pp, eps_t, qT)
            _phase2(nc, tc, qT, kT_dram, v_dram, acol, arow, wo, xres, x2_dram,
                    ones, rec_dram)
            _phase3(nc, tc, x2_dram, h2_dram, w1, b1_sb, w2, b2_bc, eps_t, out)

    _split_waits(nc)
    return nc


def _layernorm_tile(nc, pool, xt, eps_t, out_dtype=bf16):
    """token-major LN on a [128, D] f32 tile (w/b folded into weights)."""
    stats = pool.tile([128, 4, 6], f32, tag="lnstats")
    xg = xt.rearrange("p (n f) -> p n f", f=512)
    for i in range(4):
        nc.vector.bn_stats(out=stats[:, i, :], in_=xg[:, i, :])
    mv = pool.tile([128, 2], f32, tag="lnmv")
    nc.vector.bn_aggr(out=mv, in_=stats)
    rs = pool.tile([128, 1], f32, tag="lnrs")
    nc.scalar.activation(out=rs, in_=mv[:, 1:2],
                         func=mybir.ActivationFunctionType.Sqrt,
                         bias=eps_t, scale=1.0)
    nc.vector.reciprocal(out=rs, in_=rs)
    h = pool.tile([128, D], out_dtype, tag="lnh")
    nc.vector.tensor_scalar(out=h, in0=xt, scalar1=mv[:, 0:1], scalar2=rs,
                            op0=mybir.AluOpType.subtract,
                            op1=mybir.AluOpType.mult)
    return h


def _phase1(nc, tc, xp, h_dram, kT_dram, v_dram, wqkv, bq_pp, eps_t, qT):
    """LN1 over all tokens; h -> DRAM; per 512-token panel DMA-transpose
    h^T back and run K/V projections (and Q on panel 0)."""
    with (
        tc.tile_pool(name="p1w", bufs=1) as wpool,
        tc.tile_pool(name="p1", bufs=2) as sb,
        tc.tile_pool(name="p1h", bufs=2) as hpool,
        tc.tile_pool(name="p1q", bufs=2) as qwpool,
        tc.tile_pool(name="p1ps", bufs=3, space="PSUM") as psK,
        tc.tile_pool(name="p1psv", bufs=3, space="PSUM") as psV,
        tc.tile_pool(name="p1psq", bufs=2, space="PSUM") as psQ,
    ):
        bq_sb = wpool.tile([128, DT16], f32, tag="bq")
        nc.sync.dma_start(out=bq_sb, in_=bq_pp.ap())
        # resident K weights: 16 tiles [128 d, 2048]; V weights streamed
        wk_sb = []
        for dt in range(DT16):
            wkt = wpool.tile([128, D], bf16, tag=f"wk{dt}")
            nc.sync.dma_start(out=wkt,
                              in_=wqkv.ap()[dt * 128:(dt + 1) * 128, D:2 * D])
            wk_sb.append(wkt)

        for pan in range(4):
            for t in range(4):
                row0 = pan * 512 + t * 128
                xt = sb.tile([128, D], f32, tag="x")
                nc.sync.dma_start(out=xt, in_=xp.ap()[row0:row0 + 128, :])
                h = _layernorm_tile(nc, sb, xt, eps_t)
                nc.sync.dma_start(out=h_dram.ap()[row0:row0 + 128, :], in_=h)
            hT = []
            for dt in range(DT16):
                ht = hpool.tile([128, 512], bf16, tag=f"hT{dt}")
                nc.sync.dma_start_transpose(
                    out=ht,
                    in_=h_dram.ap()[pan * 512:(pan + 1) * 512,
                                    dt * 128:(dt + 1) * 128])
                hT.append(ht)
            # K projection: k^T tiles, bias dropped (cancels in softmax)
            for m in range(DT16):
                ps = psK.tile([128, 512], f32)
                for dt in range(DT16):
                    nc.tensor.matmul(ps, wk_sb[dt][:, m * 128:(m + 1) * 128],
                                     hT[dt], start=(dt == 0),
                                     stop=(dt == DT16 - 1))
                kt = sb.tile([128, 512], bf16, tag="kout")
                nc.scalar.copy(out=kt, in_=ps)
                nc.sync.dma_start(
                    out=kT_dram.ap()[m * 128:(m + 1) * 128,
                                     pan * 512:(pan + 1) * 512], in_=kt)
            # V projection (token-major), bias folded into xres on host
            for nch in range(4):
                wv = qwpool.tile([128, DT16, 512], bf16, tag="wv")
                nc.sync.dma_start(
                    out=wv,
                    in_=wqkv.ap()[:, 2 * D + nch * 512:2 * D + (nch + 1) * 512]
                    .rearrange("(dt p) f -> p dt f", p=128))
                for t in range(4):
                    ps = psV.tile([128, 512], f32)
                    for dt in range(DT16):
                        nc.tensor.matmul(ps, hT[dt][:, t * 128:(t + 1) * 128],
                                         wv[:, dt, :], start=(dt == 0),
                                         stop=(dt == DT16 - 1))
                    vt = sb.tile([128, 512], bf16, tag="vout")
                    nc.scalar.copy(out=vt, in_=ps)
                    nc.sync.dma_start(
                        out=v_dram.ap()[pan * 512 + t * 128:
                                        pan * 512 + (t + 1) * 128,
                                        nch * 512:(nch + 1) * 512], in_=vt)
            if pan == 0:
                for m in range(DT16):
                    wq = qwpool.tile([128, DT16, 128], bf16, tag="wq")
                    nc.sync.dma_start(
                        out=wq,
                        in_=wqkv.ap()[:, m * 128:(m + 1) * 128]
                        .rearrange("(dt p) f -> p dt f", p=128))
                    ps = psQ.tile([128, 512], f32)
                    for dt in range(DT16):
                        nc.tensor.matmul(ps, wq[:, dt, :], hT[dt],
                                         start=(dt == 0), stop=(dt == DT16 - 1))
                    nc.vector.tensor_scalar(out=qT[m], in0=ps,
                                            scalar1=bq_sb[:, m:m + 1],
                                            scalar2=None,
                                            op0=mybir.AluOpType.add)


def _phase2(nc, tc, qT, kT_dram, v_dram, acol, arow, wo, xres, x2_dram, ones,
            rec_dram):
    """attention (scores^T layout, softmax w/o max-sub, separable alibi:
    per-partition acol add + row-tile subtract) + o_proj + residual -> x2.

    Head-outer: K^T/V for all 16 permuted k-tiles load once per head (the
    slot-1 padded extent covers slot 0's), attnT tiles for all (h, slot)
    accumulate, then o_proj runs token-major."""
    SK = sum(KEXT)
    with (
        tc.tile_pool(name="p2wo", bufs=1) as wopool,
        tc.tile_pool(name="p2kv", bufs=2) as kvpool,
        tc.tile_pool(name="p2aq", bufs=2) as aqpool,
        tc.tile_pool(name="p2s", bufs=4) as spool,
        tc.tile_pool(name="p2at", bufs=1) as atpool,
        tc.tile_pool(name="p2o", bufs=3) as opool,
        tc.tile_pool(name="p2den", bufs=4) as denpool,
        tc.tile_pool(name="psS", bufs=2, space="PSUM") as psS,
        tc.tile_pool(name="psAV", bufs=2, space="PSUM") as psAV,
        tc.tile_pool(name="psD", bufs=2, space="PSUM") as psD,
        tc.tile_pool(name="psO", bufs=2, space="PSUM") as psO,
    ):
        acol_sb = wopool.tile([128, H * SK], f32, tag="acol")
        nc.sync.dma_start(out=acol_sb, in_=acol.ap())
        wo_sb = []
        for ht in range(DT16):
            wot = wopool.tile([128, D], bf16, tag=f"wo{ht}")
            nc.sync.dma_start(out=wot, in_=wo.ap()[ht * 128:(ht + 1) * 128, :])
            wo_sb.append(wot)

        attnT = {}
        for h in range(H):
            # all 16 permuted k-tiles for this head, one DMA each
            kt_sb = kvpool.tile([128, NKT * 128], bf16, tag="kt")
            nc.sync.dma_start(out=kt_sb,
                              in_=kT_dram.ap()[h * 128:(h + 1) * 128, :])
            v_sb = kvpool.tile([128, NKT, 128], bf16, tag="vt")
            nc.sync.dma_start(
                out=v_sb,
                in_=v_dram.ap()[:, h * 128:(h + 1) * 128]
                .rearrange("(n p) f -> p n f", p=128))
            ar_sb = aqpool.tile([128, 3, QW], f32, tag="ar")
            nc.sync.dma_start(out=ar_sb,
                              in_=arow.ap()[h].rearrange("v p f -> p v f"))

            for j in range(NSLOT):
                ext = KEXT[j]
                nown = 2 * j + 2
                pav = psAV.tile([128, QW], f32)
                pden = psD.tile([1, QW], f32)
                qslice = qT[h][:, j * QW:(j + 1) * QW]
                for p in range(ext):
                    permpos = p if p < nown else p + (4 - nown)
                    var = 1 if p == 2 * j else (2 if p == 2 * j + 1 else 0)
                    cidx = h * SK + sum(KEXT[:j]) + p
                    ps = psS.tile([128, QW], f32)
                    nc.tensor.matmul(
                        ps, kt_sb[:, permpos * 128:(permpos + 1) * 128],
                        qslice, start=True, stop=True)
                    ss = spool.tile([128, QW], f32, tag="ss")
                    nc.vector.scalar_tensor_tensor(
                        out=ss, in0=ps, scalar=acol_sb[:, cidx:cidx + 1],
                        in1=ar_sb[:, var, :],
                        op0=mybir.AluOpType.add,
                        op1=mybir.AluOpType.subtract)
                    es = spool.tile([128, QW], bf16, tag="es")
                    nc.scalar.activation(out=es, in_=ss,
                                         func=mybir.ActivationFunctionType.Exp)
                    nc.tensor.matmul(pden, ones, es,
                                     start=(p == 0), stop=(p == ext - 1))
                    nc.tensor.matmul(pav, v_sb[:, permpos, :], es,
                                     start=(p == 0), stop=(p == ext - 1))
                rec = denpool.tile([1, QW], f32, tag="rec")
                nc.vector.reciprocal(out=rec, in_=pden)
                row = rec_dram.ap()[j * H + h:j * H + h + 1, :]
                nc.sync.dma_start(out=row, in_=rec)
                recb = denpool.tile([128, QW], f32, tag="recb")
                bc = bass.AP(tensor=row.tensor, offset=row.offset,
                             ap=[[0, 128]] + list(row.ap[1:]))
                nc.gpsimd.dma_start(out=recb, in_=bc)
                at = atpool.tile([128, QW], bf16, tag=f"at{h}_{j}",
                                 name=f"at{h}_{j}")
                nc.vector.tensor_mul(out=at, in0=pav, in1=recb)
                attnT[(h, j)] = at

        for j in range(NSLOT):
            for tt in range(QW // 128):
                for dc in range(4):
                    ps = psO.tile([128, 512], f32)
                    for h in range(H):
                        nc.tensor.matmul(
                            ps, attnT[(h, j)][:, tt * 128:(tt + 1) * 128],
                            wo_sb[h][:, dc * 512:(dc + 1) * 512],
                            start=(h == 0), stop=(h == H - 1))
                    row0 = j * QW + tt * 128
                    xr = opool.tile([128, 512], f32, tag="xr")
                    nc.sync.dma_start(
                        out=xr, in_=xres.ap()[row0:row0 + 128,
                                              dc * 512:(dc + 1) * 512])
                    x2 = opool.tile([128, 512], f32, tag="x2")
                    nc.vector.tensor_add(out=x2, in0=ps, in1=xr)
                    nc.sync.dma_start(
                        out=x2_dram.ap()[row0:row0 + 128,
                                         dc * 512:(dc + 1) * 512], in_=x2)


def _phase3(nc, tc, x2_dram, h2_dram, w1, b1_sb, w2, b2_bc, eps_t, out):
    """LN2 + GELU MLP + residual on the 512 own tokens."""
    NQ = 16  # f-tiles per w2 quarter-chunk
    with (
        tc.tile_pool(name="p3", bufs=2) as sb,
        tc.tile_pool(name="p3h", bufs=1) as hpool,
        tc.tile_pool(name="p3m", bufs=1) as mpool,
        tc.tile_pool(name="p3w1", bufs=2) as w1pool,
        tc.tile_pool(name="p3w2", bufs=2) as w2pool,
        tc.tile_pool(name="p3x2", bufs=2) as x2pool,
        tc.tile_pool(name="psM1", bufs=3, space="PSUM") as psM1,
        tc.tile_pool(name="psM2", bufs=4, space="PSUM") as psM2,
    ):
        b2_sb = hpool.tile([128, D], f32, tag="b2")
        nc.sync.dma_start(out=b2_sb, in_=b2_bc.ap())
        for t in range(4):
            x2t = sb.tile([128, D], f32, tag="x2")
            nc.sync.dma_start(out=x2t,
                              in_=x2_dram.ap()[t * 128:(t + 1) * 128, :])
            h2 = _layernorm_tile(nc, sb, x2t, eps_t)
            nc.sync.dma_start(out=h2_dram.ap()[t * 128:(t + 1) * 128, :],
                              in_=h2)
        h2T = []
        for dt in range(DT16):
            ht = hpool.tile([128, QT], bf16, tag=f"h2T{dt}")
            nc.sync.dma_start_transpose(
                out=ht, in_=h2_dram.ap()[:, dt * 128:(dt + 1) * 128])
            h2T.append(ht)
        # MLP1 + gelu -> m1^T tiles [128 f, 512]
        m1 = []
        for m in range(FT64):
            w1t = w1pool.tile([128, DT16, 128], bf16, tag="w1")
            nc.sync.dma_start(
                out=w1t,
                in_=w1.ap()[:, m * 128:(m + 1) * 128]
                .rearrange("(dt p) f -> p dt f", p=128))
            ps = psM1.tile([128, QT], f32)
            for dt in range(DT16):
                nc.tensor.matmul(ps, w1t[:, dt, :], h2T[dt],
                                 start=(dt == 0), stop=(dt == DT16 - 1))
            mt = mpool.tile([128, QT], bf16, tag=f"m1_{m}")
            nc.scalar.activation(
                out=mt, in_=ps,
                func=mybir.ActivationFunctionType.Gelu_apprx_tanh,
                bias=b1_sb[:, m:m + 1], scale=1.0)
            m1.append(mt)
        # MLP2 (token-major out) + residual + b2; w2 streamed in quarter
        # chunks, 4 psum banks accumulate one t-tile each across quarters.
        for dc in range(4):
            pss = [psM2.tile([128, 512], f32, name=f"psm2_{t}", tag="psm2")
                   for t in range(4)]
            for qc in range(4):
                w2t = w2pool.tile([128, NQ, 512], bf16, tag="w2")
                nc.sync.dma_start(
                    out=w2t,
                    in_=w2.ap()[qc * NQ * 128:(qc + 1) * NQ * 128,
                                dc * 512:(dc + 1) * 512]
                    .rearrange("(ft p) f -> p ft f", p=128))
                for t in range(4):
                    for f in range(NQ):
                        ft = qc * NQ + f
                        nc.tensor.matmul(
                            pss[t], m1[ft][:, t * 128:(t + 1) * 128],
                            w2t[:, f, :],
                            start=(ft == 0), stop=(ft == FT64 - 1))
            for t in range(4):
                x2t = x2pool.tile([128, 512], f32, tag="x2rd")
                nc.sync.dma_start(
                    out=x2t, in_=x2_dram.ap()[t * 128:(t + 1) * 128,
                                              dc * 512:(dc + 1) * 512])
                s1 = x2pool.tile([128, 512], f32, tag="s1")
                nc.vector.tensor_add(out=s1, in0=pss[t], in1=x2t)
                o = x2pool.tile([128, 512], f32, tag="o")
                nc.vector.tensor_add(out=o, in0=s1,
                                     in1=b2_sb[:, dc * 512:(dc + 1) * 512])
                nc.sync.dma_start(
                    out=out.ap()[t * 128:(t + 1) * 128,
                                 dc * 512:(dc + 1) * 512], in_=o)


# ---------------------------------------------------------------------------
# host wrapper
# ---------------------------------------------------------------------------
_nc_cache = {}


def _get_nc(debug=False):
    if debug not in _nc_cache:
        _nc_cache[debug] = build_nc(debug=debug)
    return _nc_cache[debug]


def _prep_inputs(x, ln1_w, ln1_b, wqkv, bqkv, wo, bo, ln2_w, ln2_b,
                 w1, b1, w2, b2):
    slopes = _alibi_slopes(H)
    wqkv_f = (ln1_w[:, None] * wqkv).astype(np.float32)
    bqkv_f = (ln1_b @ wqkv + bqkv).astype(np.float32)
    wqkv_f[:, :D] *= QSCALE
    bqkv_f[:D] *= QSCALE
    w1_f = (ln2_w[:, None] * w1).astype(np.float32)
    b1_f = (ln2_b @ w1 + b1).astype(np.float32)

    wqkv_b = wqkv_f.astype(ml_dtypes.bfloat16)
    wo_b = wo.astype(ml_dtypes.bfloat16)
    w1_b = w1_f.astype(ml_dtypes.bfloat16)
    w2_b = w2.astype(ml_dtypes.bfloat16)

    bq_pp = bqkv_f[:D].reshape(DT16, 128).T.copy().astype(np.float32)
    b1_pp = b1_f.reshape(FT64, 128).T.copy().astype(np.float32)
    b2_bc = np.broadcast_to(b2.astype(np.float32), (128, D)).copy()
    # v-bias contributes exactly bv @ wo to the attention output
    res_const = (bo + bqkv_f[2 * D:] @ wo).astype(np.float32)
    arow = _build_arow(slopes)

    in_maps = []
    metas = []
    for c in range(NCORES):
        batch, r = divmod(c, GS)
        _, perm128, _ = _rank_structure(r)
        perm_tok = np.concatenate(
            [np.arange(t * 128, (t + 1) * 128) for t in perm128])
        xp = np.ascontiguousarray(x[batch][perm_tok]).astype(np.float32)
        xr = (xp[:QT] + res_const[None, :]).astype(np.float32)
        in_maps.append({
            "xp": xp, "xres": xr,
            "wqkv": wqkv_b, "bq_pp": bq_pp,
            "wo": wo_b, "w1": w1_b, "b1_pp": b1_pp,
            "w2": w2_b, "b2_bc": b2_bc,
            "acol": _build_acol(r, slopes), "arow": arow,
        })
        metas.append((batch, perm_tok[:QT]))
    return in_maps, metas


last_result = None


def _install_ntff_hook_shim():
    """Register the boot script's ctypes NTFF hook under the module name
    bass_utils expects, and disable artifact upload (zero-egress box)."""
    import sys as _sys
    import types
    if "antenv.axon_hooks" not in _sys.modules:
        import importlib
        tb = importlib.import_module("trn_agent_boot.trn_boot")
        hook = tb._ntff_profile_via_ctypes("/opt/axon/libaxon_pjrt.so")
        mod = types.ModuleType("antenv.axon_hooks")
        mod.get_axon_ntff_profile_hook = lambda: hook
        _sys.modules["antenv.axon_hooks"] = mod
    import concourse.bass_utils as bu
    bu.upload_artifacts = lambda tmpdir: "(upload disabled)"


def kernel(**inputs):
    global last_result
    args = {k: np.asarray(v, dtype=np.float32) for k, v in inputs.items()}
    in_maps, metas = _prep_inputs(
        args["x"], args["ln1_w"], args["ln1_b"], args["wqkv"], args["bqkv"],
        args["wo"], args["bo"], args["ln2_w"], args["ln2_b"],
        args["w1"], args["b1"], args["w2"], args["b2"])
    nc = _get_nc()
    kwargs = {}
    if os.environ.get("KBENCH_TRACE"):
        _install_ntff_hook_shim()
        kwargs = dict(trace=True, trace_cores=list(range(NCORES)))
    res = run_bass_kernel_spmd(nc, in_maps, core_ids=list(range(NCORES)),
                               **kwargs)
    last_result = res
    out = np.empty((B, S, D), dtype=np.float32)
    for c in range(NCORES):
        batch, tok = metas[c]
        out[batch, tok] = res.results[c]["out"]
    return out



# revision 4
# speedup vs baseline: 1.1231x; 1.1231x over previous
"""Bloom transformer block on 8 Trainium2 NeuronCores.

Sharding: core c handles batch c//4 and 512 of its 2048 tokens (two
causally-balanced 256-token q-tiles {r, r+4}, r = c%4).  Each core computes
LN1 + Q/K/V projections for ONLY its own 512 tokens; K^T and V slices are
then shared within each 4-core batch group via chunked AllGather collectives
(one per 4-head group, interleaved with compute so the cc stream hides under
projection/attention).  Attention/o_proj/MLP run on own tokens only.  The
diagonal attention k-tiles (own tokens) are read from local SBUF at fixed
program positions, so the triangular causal masks stay compile-time; all
remaining rank-dependence lives in host-prepared acol data (alibi columns +
NEG masking of invalid gathered tiles).  The device program is identical
across cores (SPMD).  The host concatenates the per-core output slices.

Exact math shortcuts: the k-projection bias adds a per-query constant to
every score row, so it cancels in softmax and is dropped; the v-projection
bias adds exactly bv to each attention output (probs sum to 1), so bv @ wo
is folded into the host-prepared residual.  LN affine params are folded
into the following matmul weights; 1/sqrt(HD) is folded into Wq.
"""

import math
import os

import numpy as np
import ml_dtypes

import concourse.bass as bass
import concourse.tile as tile
from concourse import mybir
from concourse.bass_utils import run_bass_kernel_spmd

B, S, D, H = 2, 2048, 2048, 16
HD = D // H          # 128
FF = 4 * D           # 8192
EPS = 1e-5
NCORES = 8
GS = 4               # cores per batch (group size)
QT = S // GS         # own tokens per core = 512
NEG = -1.0e9
QW = 256             # q-tile width (tokens) in attention
NSLOT = QT // QW     # 2 slots per core
NKT = S // 128       # 16 k-tiles
DT16 = D // 128      # 16 feature tiles of 128
FT64 = FF // 128     # 64 FF tiles
QSCALE = 1.0 / math.sqrt(HD)
GROUPS = [[0, 1, 2, 3], [4, 5, 6, 7]]
# per-slot read extents (in 128-wide k-tiles): 2 local diag + gathered
KEXT = [8, 16]
SK = sum(KEXT)       # 24 acol columns per head

f32 = mybir.dt.float32
bf16 = mybir.dt.bfloat16


def _alibi_slopes(num_heads):
    closest = 2 ** math.floor(math.log2(num_heads))
    base = 2.0 ** (-(2.0 ** (-(math.log2(closest) - 3))))
    powers = np.arange(1, 1 + closest, dtype=np.float64)
    slopes = base ** powers
    if closest != num_heads:
        extra_base = 2.0 ** (-(2.0 ** (-(math.log2(2 * closest) - 3))))
        num_rem = min(closest, num_heads - closest)
        extra_powers = np.arange(1, 1 + 2 * num_rem, 2, dtype=np.float64)
        slopes = np.concatenate([slopes, extra_base ** extra_powers])
    return slopes.astype(np.float32)


# ---------------------------------------------------------------------------
# wait-split post-pass: this walrus build supports a single sync-wait per
# instruction; excess waits move onto preceding NoOps on the same engine.
# ---------------------------------------------------------------------------
_ctr = [0]


def _split_waits(nc, maxw=1):
    for f in nc.m.functions:
        for bb in f.blocks:
            out = []
            changed = False
            for ins in bb.instructions:
                si = ins.sync_info
                waits = list(si.on_wait) if (si and si.on_wait) else []
                if len(waits) > maxw:
                    head, keep = waits[:-maxw], waits[-maxw:]
                    for w in head:
                        _ctr[0] += 1
                        nop = mybir.InstNoOp(name=f"I-waitsplit-{_ctr[0]}")
                        nop.engine = ins.engine
                        nop.sync_info = mybir.SyncInfo(on_wait=[w], on_update=[])
                        out.append(nop)
                    si.on_wait = keep
                    changed = True
                out.append(ins)
            if changed:
                bb.instructions = out
    return nc


# ---------------------------------------------------------------------------
# attention p-loop read map (program-level, rank-independent)
# entries: ("loc", col0) local kt_own column, or ("rr", rank, col0) gathered
# tile kt_sb[:, rank, col0:col0+128].  var: 0 plain, 1/2 diag triangle.
# ---------------------------------------------------------------------------
def _read_map():
    plan = []                     # per slot: list of (src, var)
    s0 = [(("loc", 0), 1), (("loc", 128), 2)]
    for rr in range(3):
        for half in range(2):
            s0.append((("rr", rr, half * 128), 0))
    plan.append(s0)
    s1 = [(("loc", 256), 1), (("loc", 384), 2)]
    for rr in range(4):
        for half in range(2):
            s1.append((("rr", rr, half * 128), 0))
    for rr in range(3):
        for half in range(2):
            s1.append((("rr", rr, 256 + half * 128), 0))
    plan.append(s1)
    return plan


READ_MAP = _read_map()


def _build_acol(r, slopes):
    """Per-core [128, H*SK] f32 column tiles: value slope_h*(k_orig - q_base)
    per in-tile k position, or NEG for masked (beyond-causal or
    locally-handled) gathered tiles."""
    out = np.empty((128, H * SK), dtype=np.float32)
    kk = np.arange(128, dtype=np.float64)
    for h in range(H):
        for j in range(NSLOT):
            qtile = r + 4 * j               # orig 256-q-tile index
            qbase = qtile * QW
            off = h * SK + sum(KEXT[:j])
            for p, (src, var) in enumerate(READ_MAP[j]):
                if src[0] == "loc":
                    # own diag 128-k-tile: src[1] is the kt_own column
                    # offset; in-slot k offset is src[1] % 256
                    out[:, off + p] = slopes[h] * (src[1] % 256 + kk)
                else:
                    _, rr, col0 = src
                    # gathered tile: rank rr's token col0 (0:256 -> orig
                    # 256-tile rr; 256:512 -> orig rr+4)
                    otile = rr if col0 < 256 else rr + 4
                    k0 = otile * QW + (col0 % 256)
                    # active iff strictly before own q-tile (diagonal is
                    # handled locally; beyond-causal masked)
                    if otile < qtile:
                        out[:, off + p] = slopes[h] * (k0 + kk - qbase)
                    else:
                        out[:, off + p] = NEG
    return out


def _build_arow(slopes):
    """Shared [H, 3, 128, QW] f32 row tiles (subtracted from scores):
    var 0 = plain slope_h*qq; var 1/2 = plain + 1e9 on the causally-masked
    cells of the two own-diagonal k-tiles (kk > qq, 128+kk > qq)."""
    out = np.empty((H, 3, 128, QW), dtype=np.float32)
    kk = np.arange(128)
    qq = np.arange(QW)
    m0 = (kk[:, None] > qq[None, :]).astype(np.float32) * (-NEG)
    m1 = ((kk[:, None] + 128) > qq[None, :]).astype(np.float32) * (-NEG)
    for h in range(H):
        plain = np.broadcast_to(slopes[h] * qq[None, :].astype(np.float32),
                                (128, QW))
        out[h, 0] = plain
        out[h, 1] = plain + m0
        out[h, 2] = plain + m1
    return out


# ---------------------------------------------------------------------------
# device program (identical for all cores)
# ---------------------------------------------------------------------------
def build_nc():
    nc = bass.Bass(target_bir_lowering=False)

    xp = nc.dram_tensor("xp", [QT, D], f32, kind="ExternalInput")
    xres = nc.dram_tensor("xres", [QT, D], f32, kind="ExternalInput")
    wqkv = nc.dram_tensor("wqkv", [D, 3 * D], bf16, kind="ExternalInput")
    bq_pp = nc.dram_tensor("bq_pp", [128, DT16], f32, kind="ExternalInput")
    wo = nc.dram_tensor("wo", [D, D], bf16, kind="ExternalInput")
    w1 = nc.dram_tensor("w1", [D, FF], bf16, kind="ExternalInput")
    b1_pp = nc.dram_tensor("b1_pp", [128, FT64], f32, kind="ExternalInput")
    w2 = nc.dram_tensor("w2", [FF, D], bf16, kind="ExternalInput")
    b2_bc = nc.dram_tensor("b2_bc", [128, D], f32, kind="ExternalInput")
    acol = nc.dram_tensor("acol", [128, H * SK], f32, kind="ExternalInput")
    arow = nc.dram_tensor("arow", [H, 3, 128, QW], f32, kind="ExternalInput")

    out = nc.dram_tensor("out", [QT, D], f32, kind="ExternalOutput")

    h_dram = nc.dram_tensor("h_dram", [QT, D], bf16, kind="Internal")
    kT_local = nc.dram_tensor("kT_local", [D, QT], bf16, kind="Internal")
    v_chunks = nc.dram_tensor("v_chunks", [4, QT, 512], bf16, kind="Internal")
    kT_full = [nc.dram_tensor(f"kT_full{g}", [4 * 512, QT], bf16,
                              kind="Internal") for g in range(4)]
    v_full = [nc.dram_tensor(f"v_full{g}", [4 * QT, 512], bf16,
                             kind="Internal") for g in range(4)]
    x2_dram = nc.dram_tensor("x2_dram", [QT, D], f32, kind="Internal")
    h2_dram = nc.dram_tensor("h2_dram", [QT, D], bf16, kind="Internal")
    rec_dram = nc.dram_tensor("rec_dram", [NSLOT * H, QW], f32, kind="Internal")

    with tile.TileContext(nc) as tc:
        with tc.tile_pool(name="persist", bufs=1) as pp:
            ones = pp.tile([128, 1], bf16, tag="ones")
            nc.vector.memset(ones, 1.0)
            eps_t = pp.tile([128, 1], f32, tag="eps")
            nc.vector.memset(eps_t, EPS)
            b1_sb = pp.tile([128, FT64], f32, tag="b1")
            nc.sync.dma_start(out=b1_sb, in_=b1_pp.ap())

            with tc.tile_pool(name="attn_persist", bufs=1) as app:
                qT = [app.tile([128, QT], bf16, tag=f"qT{m}", name=f"qT{m}")
                      for m in range(DT16)]
                kt_own = [app.tile([128, QT], bf16, tag=f"ktown{m}",
                                   name=f"ktown{m}") for m in range(DT16)]
                # v_own[nch][t]: [128 tok, 512 feat]
                v_own = [[app.tile([128, 512], bf16, tag=f"vown{n}_{t}",
                                   name=f"vown{n}_{t}") for t in range(4)]
                         for n in range(4)]

                _phase1(nc, tc, xp, h_dram, kT_local, v_chunks, kT_full,
                        v_full, wqkv, bq_pp, eps_t, qT, kt_own, v_own)
                _phase2(nc, tc, qT, kt_own, v_own, kT_full, v_full, acol,
                        arow, wo, xres, x2_dram, ones, rec_dram)
            _phase3(nc, tc, x2_dram, h2_dram, w1, b1_sb, w2, b2_bc, eps_t, out)

    _split_waits(nc)
    return nc


def _layernorm_tile(nc, pool, xt, eps_t, out_dtype=bf16):
    """token-major LN on a [128, D] f32 tile (w/b folded into weights)."""
    stats = pool.tile([128, 4, 6], f32, tag="lnstats")
    xg = xt.rearrange("p (n f) -> p n f", f=512)
    for i in range(4):
        nc.vector.bn_stats(out=stats[:, i, :], in_=xg[:, i, :])
    mv = pool.tile([128, 2], f32, tag="lnmv")
    nc.vector.bn_aggr(out=mv, in_=stats)
    rs = pool.tile([128, 1], f32, tag="lnrs")
    nc.scalar.activation(out=rs, in_=mv[:, 1:2],
                         func=mybir.ActivationFunctionType.Sqrt,
                         bias=eps_t, scale=1.0)
    nc.vector.reciprocal(out=rs, in_=rs)
    h = pool.tile([128, D], out_dtype, tag="lnh")
    nc.vector.tensor_scalar(out=h, in0=xt, scalar1=mv[:, 0:1], scalar2=rs,
                            op0=mybir.AluOpType.subtract,
                            op1=mybir.AluOpType.mult)
    return h


def _phase1(nc, tc, xp, h_dram, kT_local, v_chunks, kT_full, v_full,
            wqkv, bq_pp, eps_t, qT, kt_own, v_own):
    """LN1 on own 512 tokens; K proj (-> gathers), Q proj, V proj
    (-> gathers).  K/V outputs stay SBUF-resident for the diagonal
    attention tiles and are DMA'd to DRAM for the group AllGathers."""
    with (
        tc.tile_pool(name="p1w", bufs=1) as wpool,
        tc.tile_pool(name="p1", bufs=2) as sb,
        tc.tile_pool(name="p1h", bufs=1) as hpool,
        tc.tile_pool(name="p1q", bufs=2) as qwpool,
        tc.tile_pool(name="p1ps", bufs=3, space="PSUM") as psK,
        tc.tile_pool(name="p1psv", bufs=3, space="PSUM") as psV,
        tc.tile_pool(name="p1psq", bufs=2, space="PSUM") as psQ,
    ):
        bq_sb = wpool.tile([128, DT16], f32, tag="bq")
        nc.sync.dma_start(out=bq_sb, in_=bq_pp.ap())
        # resident K weights: 16 tiles [128 d, 2048]
        wk_sb = []
        for dt in range(DT16):
            wkt = wpool.tile([128, D], bf16, tag=f"wk{dt}")
            nc.sync.dma_start(out=wkt,
                              in_=wqkv.ap()[dt * 128:(dt + 1) * 128, D:2 * D])
            wk_sb.append(wkt)

        for t in range(4):
            row0 = t * 128
            xt = sb.tile([128, D], f32, tag="x")
            nc.sync.dma_start(out=xt, in_=xp.ap()[row0:row0 + 128, :])
            h = _layernorm_tile(nc, sb, xt, eps_t)
            nc.sync.dma_start(out=h_dram.ap()[row0:row0 + 128, :], in_=h)
        hT = []
        for dt in range(DT16):
            ht = hpool.tile([128, QT], bf16, tag=f"hT{dt}")
            nc.sync.dma_start_transpose(
                out=ht, in_=h_dram.ap()[:, dt * 128:(dt + 1) * 128])
            hT.append(ht)

        # K projection -> kt_own (resident) + kT_local + chunked gathers
        for m in range(DT16):
            ps = psK.tile([128, QT], f32)
            for dt in range(DT16):
                nc.tensor.matmul(ps, wk_sb[dt][:, m * 128:(m + 1) * 128],
                                 hT[dt], start=(dt == 0), stop=(dt == DT16 - 1))
            nc.scalar.copy(out=kt_own[m], in_=ps)
            nc.sync.dma_start(out=kT_local.ap()[m * 128:(m + 1) * 128, :],
                              in_=kt_own[m])
            if m % 4 == 3:
                g = m // 4
                nc.gpsimd.collective_compute(
                    "AllGather", mybir.AluOpType.bypass,
                    replica_groups=GROUPS,
                    ins=[kT_local.ap()[g * 512:(g + 1) * 512, :]],
                    outs=[kT_full[g].ap()])

        # Q projection (heads 0-3 early so attention can start)
        def qproj(m):
            wq = qwpool.tile([128, DT16, 128], bf16, tag="wq")
            nc.sync.dma_start(
                out=wq,
                in_=wqkv.ap()[:, m * 128:(m + 1) * 128]
                .rearrange("(dt p) f -> p dt f", p=128))
            ps = psQ.tile([128, QT], f32)
            for dt in range(DT16):
                nc.tensor.matmul(ps, wq[:, dt, :], hT[dt],
                                 start=(dt == 0), stop=(dt == DT16 - 1))
            nc.vector.tensor_scalar(out=qT[m], in0=ps,
                                    scalar1=bq_sb[:, m:m + 1],
                                    scalar2=None,
                                    op0=mybir.AluOpType.add)

        for m in range(4):
            qproj(m)

        # V projection -> v_own (resident) + v_chunks + chunked gathers
        for nch in range(4):
            wv = qwpool.tile([128, DT16, 512], bf16, tag="wv")
            nc.sync.dma_start(
                out=wv,
                in_=wqkv.ap()[:, 2 * D + nch * 512:2 * D + (nch + 1) * 512]
                .rearrange("(dt p) f -> p dt f", p=128))
            for t in range(4):
                ps = psV.tile([128, 512], f32)
                for dt in range(DT16):
                    nc.tensor.matmul(ps, hT[dt][:, t * 128:(t + 1) * 128],
                                     wv[:, dt, :], start=(dt == 0),
                                     stop=(dt == DT16 - 1))
                nc.scalar.copy(out=v_own[nch][t], in_=ps)
                nc.sync.dma_start(
                    out=v_chunks.ap()[nch, t * 128:(t + 1) * 128, :],
                    in_=v_own[nch][t])
            nc.gpsimd.collective_compute(
                "AllGather", mybir.AluOpType.bypass,
                replica_groups=GROUPS,
                ins=[v_chunks.ap()[nch]],
                outs=[v_full[nch].ap()])

        for m in range(4, DT16):
            qproj(m)


def _phase2(nc, tc, qT, kt_own, v_own, kT_full, v_full, acol, arow, wo,
            xres, x2_dram, ones, rec_dram):
    """attention (scores^T layout, softmax w/o max-sub, separable alibi:
    per-partition acol add + row-tile subtract) + o_proj + residual -> x2.

    Per head: 2 local diag k-tiles per slot from SBUF (fixed triangle
    masks), gathered tiles from kT_full/v_full (rank-dependent NEG masking
    in acol data)."""
    with (
        tc.tile_pool(name="p2wo", bufs=1) as wopool,
        tc.tile_pool(name="p2kv", bufs=2) as kvpool,
        tc.tile_pool(name="p2aq", bufs=2) as aqpool,
        tc.tile_pool(name="p2s", bufs=4) as spool,
        tc.tile_pool(name="p2at", bufs=1) as atpool,
        tc.tile_pool(name="p2o", bufs=3) as opool,
        tc.tile_pool(name="p2den", bufs=4) as denpool,
        tc.tile_pool(name="psS", bufs=2, space="PSUM") as psS,
        tc.tile_pool(name="psAV", bufs=2, space="PSUM") as psAV,
        tc.tile_pool(name="psD", bufs=2, space="PSUM") as psD,
        tc.tile_pool(name="psO", bufs=2, space="PSUM") as psO,
    ):
        acol_sb = wopool.tile([128, H * SK], f32, tag="acol")
        nc.sync.dma_start(out=acol_sb, in_=acol.ap())
        wo_sb = []
        for ht in range(DT16):
            wot = wopool.tile([128, D], bf16, tag=f"wo{ht}")
            nc.sync.dma_start(out=wot, in_=wo.ap()[ht * 128:(ht + 1) * 128, :])
            wo_sb.append(wot)

        attnT = {}
        for h in range(H):
            g, hh = divmod(h, 4)
            # gathered kT for this head: 4 rank tiles [128, 512]
            kt_sb = kvpool.tile([128, 4, QT], bf16, tag="kt")
            for rr in range(4):
                nc.sync.dma_start(
                    out=kt_sb[:, rr, :],
                    in_=kT_full[g].ap()[rr * 512 + hh * 128:
                                        rr * 512 + (hh + 1) * 128, :])
            # gathered v for this head: [128 tok-part, 16 tok-tile, 128 hd]
            v_sb = kvpool.tile([128, NKT, 128], bf16, tag="vt")
            nc.sync.dma_start(
                out=v_sb,
                in_=v_full[g].ap()[:, hh * 128:(hh + 1) * 128]
                .rearrange("(n p) f -> p n f", p=128))
            ar_sb = aqpool.tile([128, 3, QW], f32, tag="ar")
            nc.sync.dma_start(out=ar_sb,
                              in_=arow.ap()[h].rearrange("v p f -> p v f"))

            for j in range(NSLOT):
                ext = KEXT[j]
                pav = psAV.tile([128, QW], f32)
                pden = psD.tile([1, QW], f32)
                qslice = qT[h][:, j * QW:(j + 1) * QW]
                for p, (src, var) in enumerate(READ_MAP[j]):
                    if src[0] == "loc":
                        ksrc = kt_own[h][:, src[1]:src[1] + 128]
                        # own token 128-tile index within own 512
                        vtile = v_own[g][src[1] // 128][:, hh * 128:
                                                        (hh + 1) * 128]
                    else:
                        _, rr, col0 = src
                        ksrc = kt_sb[:, rr, col0:col0 + 128]
                        vtile = v_sb[:, rr * 4 + col0 // 128, :]
                    cidx = h * SK + sum(KEXT[:j]) + p
                    ps = psS.tile([128, QW], f32)
                    nc.tensor.matmul(ps, ksrc, qslice, start=True, stop=True)
                    ss = spool.tile([128, QW], f32, tag="ss")
                    nc.vector.scalar_tensor_tensor(
                        out=ss, in0=ps, scalar=acol_sb[:, cidx:cidx + 1],
                        in1=ar_sb[:, var, :],
                        op0=mybir.AluOpType.add,
                        op1=mybir.AluOpType.subtract)
                    es = spool.tile([128, QW], bf16, tag="es")
                    nc.scalar.activation(out=es, in_=ss,
                                         func=mybir.ActivationFunctionType.Exp)
                    nc.tensor.matmul(pden, ones, es,
                                     start=(p == 0), stop=(p == ext - 1))
                    nc.tensor.matmul(pav, vtile, es,
                                     start=(p == 0), stop=(p == ext - 1))
                rec = denpool.tile([1, QW], f32, tag="rec")
                nc.vector.reciprocal(out=rec, in_=pden)
                row = rec_dram.ap()[j * H + h:j * H + h + 1, :]
                nc.sync.dma_start(out=row, in_=rec)
                recb = denpool.tile([128, QW], f32, tag="recb")
                bc = bass.AP(tensor=row.tensor, offset=row.offset,
                             ap=[[0, 128]] + list(row.ap[1:]))
                nc.gpsimd.dma_start(out=recb, in_=bc)
                at = atpool.tile([128, QW], bf16, tag=f"at{h}_{j}",
                                 name=f"at{h}_{j}")
                nc.vector.tensor_mul(out=at, in0=pav, in1=recb)
                attnT[(h, j)] = at

        for j in range(NSLOT):
            for tt in range(QW // 128):
                for dc in range(4):
                    ps = psO.tile([128, 512], f32)
                    for h in range(H):
                        nc.tensor.matmul(
                            ps, attnT[(h, j)][:, tt * 128:(tt + 1) * 128],
                            wo_sb[h][:, dc * 512:(dc + 1) * 512],
                            start=(h == 0), stop=(h == H - 1))
                    row0 = j * QW + tt * 128
                    xr = opool.tile([128, 512], f32, tag="xr")
                    nc.sync.dma_start(
                        out=xr, in_=xres.ap()[row0:row0 + 128,
                                              dc * 512:(dc + 1) * 512])
                    x2 = opool.tile([128, 512], f32, tag="x2")
                    nc.vector.tensor_add(out=x2, in0=ps, in1=xr)
                    nc.sync.dma_start(
                        out=x2_dram.ap()[row0:row0 + 128,
                                         dc * 512:(dc + 1) * 512], in_=x2)


def _phase3(nc, tc, x2_dram, h2_dram, w1, b1_sb, w2, b2_bc, eps_t, out):
    """LN2 + GELU MLP + residual on the 512 own tokens."""
    NQ = 16  # f-tiles per w2 quarter-chunk
    with (
        tc.tile_pool(name="p3", bufs=2) as sb,
        tc.tile_pool(name="p3h", bufs=1) as hpool,
        tc.tile_pool(name="p3m", bufs=1) as mpool,
        tc.tile_pool(name="p3w1", bufs=2) as w1pool,
        tc.tile_pool(name="p3w2", bufs=2) as w2pool,
        tc.tile_pool(name="p3x2", bufs=2) as x2pool,
        tc.tile_pool(name="psM1", bufs=3, space="PSUM") as psM1,
        tc.tile_pool(name="psM2", bufs=4, space="PSUM") as psM2,
    ):
        b2_sb = hpool.tile([128, D], f32, tag="b2")
        nc.sync.dma_start(out=b2_sb, in_=b2_bc.ap())
        for t in range(4):
            x2t = sb.tile([128, D], f32, tag="x2")
            nc.sync.dma_start(out=x2t,
                              in_=x2_dram.ap()[t * 128:(t + 1) * 128, :])
            h2 = _layernorm_tile(nc, sb, x2t, eps_t)
            nc.sync.dma_start(out=h2_dram.ap()[t * 128:(t + 1) * 128, :],
                              in_=h2)
        h2T = []
        for dt in range(DT16):
            ht = hpool.tile([128, QT], bf16, tag=f"h2T{dt}")
            nc.sync.dma_start_transpose(
                out=ht, in_=h2_dram.ap()[:, dt * 128:(dt + 1) * 128])
            h2T.append(ht)
        # MLP1 + gelu -> m1^T tiles [128 f, 512]
        m1 = []
        for m in range(FT64):
            w1t = w1pool.tile([128, DT16, 128], bf16, tag="w1")
            nc.sync.dma_start(
                out=w1t,
                in_=w1.ap()[:, m * 128:(m + 1) * 128]
                .rearrange("(dt p) f -> p dt f", p=128))
            ps = psM1.tile([128, QT], f32)
            for dt in range(DT16):
                nc.tensor.matmul(ps, w1t[:, dt, :], h2T[dt],
                                 start=(dt == 0), stop=(dt == DT16 - 1))
            mt = mpool.tile([128, QT], bf16, tag=f"m1_{m}")
            nc.scalar.activation(
                out=mt, in_=ps,
                func=mybir.ActivationFunctionType.Gelu_apprx_tanh,
                bias=b1_sb[:, m:m + 1], scale=1.0)
            m1.append(mt)
        # MLP2 (token-major out) + residual + b2; w2 streamed in quarter
        # chunks, 4 psum banks accumulate one t-tile each across quarters.
        for dc in range(4):
            pss = [psM2.tile([128, 512], f32, name=f"psm2_{t}", tag="psm2")
                   for t in range(4)]
            for qc in range(4):
                w2t = w2pool.tile([128, NQ, 512], bf16, tag="w2")
                nc.sync.dma_start(
                    out=w2t,
                    in_=w2.ap()[qc * NQ * 128:(qc + 1) * NQ * 128,
                                dc * 512:(dc + 1) * 512]
                    .rearrange("(ft p) f -> p ft f", p=128))
                for t in range(4):
                    for f in range(NQ):
                        ft = qc * NQ + f
                        nc.tensor.matmul(
                            pss[t], m1[ft][:, t * 128:(t + 1) * 128],
                            w2t[:, f, :],
                            start=(ft == 0), stop=(ft == FT64 - 1))
            for t in range(4):
                x2t = x2pool.tile([128, 512], f32, tag="x2rd")
                nc.sync.dma_start(
                    out=x2t, in_=x2_dram.ap()[t * 128:(t + 1) * 128,
                                              dc * 512:(dc + 1) * 512])
                s1 = x2pool.tile([128, 512], f32, tag="s1")
                nc.vector.tensor_add(out=s1, in0=pss[t], in1=x2t)
                o = x2pool.tile([128, 512], f32, tag="o")
                nc.vector.tensor_add(out=o, in0=s1,
                                     in1=b2_sb[:, dc * 512:(dc + 1) * 512])
                nc.sync.dma_start(
                    out=out.ap()[t * 128:(t + 1) * 128,
                                 dc * 512:(dc + 1) * 512], in_=o)


# ---------------------------------------------------------------------------
# host wrapper
# ---------------------------------------------------------------------------
_nc_cache = {}


def _get_nc():
    if "nc" not in _nc_cache:
        _nc_cache["nc"] = build_nc()
    return _nc_cache["nc"]


def _own_tokens(r):
    return np.concatenate([np.arange(r * QW, (r + 1) * QW),
                           np.arange((r + 4) * QW, (r + 5) * QW)])


def _prep_inputs(x, ln1_w, ln1_b, wqkv, bqkv, wo, bo, ln2_w, ln2_b,
                 w1, b1, w2, b2):
    slopes = _alibi_slopes(H)
    wqkv_f = (ln1_w[:, None] * wqkv).astype(np.float32)
    bqkv_f = (ln1_b @ wqkv + bqkv).astype(np.float32)
    wqkv_f[:, :D] *= QSCALE
    bqkv_f[:D] *= QSCALE
    w1_f = (ln2_w[:, None] * w1).astype(np.float32)
    b1_f = (ln2_b @ w1 + b1).astype(np.float32)

    wqkv_b = wqkv_f.astype(ml_dtypes.bfloat16)
    wo_b = wo.astype(ml_dtypes.bfloat16)
    w1_b = w1_f.astype(ml_dtypes.bfloat16)
    w2_b = w2.astype(ml_dtypes.bfloat16)

    bq_pp = bqkv_f[:D].reshape(DT16, 128).T.copy().astype(np.float32)
    b1_pp = b1_f.reshape(FT64, 128).T.copy().astype(np.float32)
    b2_bc = np.broadcast_to(b2.astype(np.float32), (128, D)).copy()
    # v-bias contributes exactly bv @ wo to the attention output
    res_const = (bo + bqkv_f[2 * D:] @ wo).astype(np.float32)
    arow = _build_arow(slopes)

    in_maps = []
    metas = []
    for c in range(NCORES):
        batch, r = divmod(c, GS)
        tok = _own_tokens(r)
        xp = np.ascontiguousarray(x[batch][tok]).astype(np.float32)
        xr = (xp + res_const[None, :]).astype(np.float32)
        in_maps.append({
            "xp": xp, "xres": xr,
            "wqkv": wqkv_b, "bq_pp": bq_pp,
            "wo": wo_b, "w1": w1_b, "b1_pp": b1_pp,
            "w2": w2_b, "b2_bc": b2_bc,
            "acol": _build_acol(r, slopes), "arow": arow,
        })
        metas.append((batch, tok))
    return in_maps, metas


last_result = None


def _install_ntff_hook_shim():
    """Register the boot script's ctypes NTFF hook under the module name
    bass_utils expects, and disable artifact upload (zero-egress box)."""
    import sys as _sys
    import types
    if "antenv.axon_hooks" not in _sys.modules:
        import importlib
        tb = importlib.import_module("trn_agent_boot.trn_boot")
        hook = tb._ntff_profile_via_ctypes("/opt/axon/libaxon_pjrt.so")
        mod = types.ModuleType("antenv.axon_hooks")
        mod.get_axon_ntff_profile_hook = lambda: hook
        _sys.modules["antenv.axon_hooks"] = mod
    import concourse.bass_utils as bu
    bu.upload_artifacts = lambda tmpdir: "(upload disabled)"


def kernel(**inputs):
    global last_result
    args = {k: np.asarray(v, dtype=np.float32) for k, v in inputs.items()}
    in_maps, metas = _prep_inputs(
        args["x"], args["ln1_w"], args["ln1_b"], args["wqkv"], args["bqkv"],
        args["wo"], args["bo"], args["ln2_w"], args["ln2_b"],
        args["w1"], args["b1"], args["w2"], args["b2"])
    nc = _get_nc()
    kwargs = {}
    if os.environ.get("KBENCH_TRACE"):
        _install_ntff_hook_shim()
        kwargs = dict(trace=True,
                      trace_cores=[int(c) for c in
                                   os.environ.get("KBENCH_TRACE_CORES",
                                                  "0").split(",")])
    res = run_bass_kernel_spmd(nc, in_maps, core_ids=list(range(NCORES)),
                               **kwargs)
    last_result = res
    out = np.empty((B, S, D), dtype=np.float32)
    for c in range(NCORES):
        batch, tok = metas[c]
        out[batch, tok] = res.results[c]["out"]
    return out
